# revision 8
# baseline (speedup 1.0000x reference)
"""Segment-reduce contrastive loss kernel for Trainium2 (8 NeuronCores).

Strategy (data-parallel over batch, per sharding hint):
  - Each of the 8 cores gets one batch element (fs/ft: [512, 16384] f32).
  - On-device per core: per-class channel sums for features_s/features_t
    computed as one-hot matmuls on the tensor engine. Features arrive
    channel-major, so each [128pix x 128ch] block is PE-transposed first
    (pixels must sit on the partition/contraction dim).
  - Per-class partial sums [19, 512] x2 are DMA'd out; the host sums the
    8 cores' partials (the "all-reduce"), computes counts, normalizes and
    does the tiny 19x19 contrastive logsumexp in numpy.

The segment matmuls use only 19 of 128 PE columns, so groups are
round-robined across 4 PSUM col-group slices (base partition 32*j) to get
sub-array-level concurrency; the 4 slices are summed on the host.
"""

import sys

for _p in ("/opt/trn_rl_repo",):
    if _p not in sys.path:
        sys.path.insert(0, _p)

from contextlib import ExitStack

import numpy as np

import concourse.bass as bass
import concourse.mybir as mybir
from concourse import bacc, tile
from concourse.bass_utils import run_bass_kernel_spmd

NUM_CLASSES = 19
TEMP = 0.1
EPS = 1e-12

B, C, H, W = 8, 512, 128, 128
HW = H * W
N_CORES = 8
P = 128
NCG = 4  # PSUM col-groups used round-robin by the segment matmuls
F32 = mybir.dt.float32


def build_nc(C_=C, HW_=HW, super_pix=1024):
    NCH = C_ // P        # channel blocks
    NG = HW_ // P        # pixel groups of 128
    GPS = super_pix // P # groups per superchunk
    NS = HW_ // super_pix
    assert NG % NCG == 0 and NG >= 2 * NCG

    nc = bacc.Bacc()
    fs = nc.declare_dram_parameter("fs", [C_, HW_], F32, isOutput=False)
    ft = nc.declare_dram_parameter("ft", [C_, HW_], F32, isOutput=False)
    # misc: [identity 128 | iota 19 | labT NG] packed along the free dim so
    # the consts arrive in ONE DMA (multiple DMA-completion sems on one
    # consumer instruction overflow walrus's per-instruction sync slots).
    misc = nc.declare_dram_parameter("misc", [P, P + NUM_CLASSES + NG], F32, isOutput=False)
    out_s = nc.declare_dram_parameter(
        "sums_s", [NCG * NUM_CLASSES, C_], F32, isOutput=True
    )
    out_t = nc.declare_dram_parameter(
        "sums_t", [NCG * NUM_CLASSES, C_], F32, isOutput=True
    )

    srcs = {"s": fs, "t": ft}
    outs = {"s": out_s, "t": out_t}

    with ExitStack() as ctx:
        tc = ctx.enter_context(tile.TileContext(nc))
        const_pool = ctx.enter_context(tc.tile_pool(name="const", bufs=1))
        nat_pool = ctx.enter_context(tc.tile_pool(name="nat", bufs=2))
        psumT_pool = ctx.enter_context(tc.tile_pool(name="psumT", bufs=2, space="PSUM"))
        acc_pool = ctx.enter_context(tc.tile_pool(name="acc", bufs=1, space="PSUM"))
        sbT_pool = ctx.enter_context(tc.tile_pool(name="sbT", bufs=3))
        oh_pool = ctx.enter_context(tc.tile_pool(name="oh", bufs=4))
        outp_pool = ctx.enter_context(tc.tile_pool(name="outp", bufs=1))

        misc_sb = const_pool.tile([P, P + NUM_CLASSES + NG], F32, tag="misc")
        nc.sync.dma_start(misc_sb[:], misc[:])
        ident = misc_sb[:, 0:P]
        iota = misc_sb[:, P : P + NUM_CLASSES]
        lab_sb = misc_sb[:, P + NUM_CLASSES : P + NUM_CLASSES + NG]

        acc = {
            t: acc_pool.tile([P, C_], F32, tag=f"acc_{t}", name=f"acc_{t}")
            for t in ("s", "t")
        }

        # Warm-up transpose reading only the const tile: pre-pays the misc
        # DMA wait on PE, so the first real transpose needs just one wait
        # (walrus allows a single embedded sync-wait per instruction).
        warm = psumT_pool.tile([P, P], F32, tag="pT_s", name="warm")
        nc.tensor.transpose(warm[:, 0:P], ident, ident)

        for j in range(NS):
            nat = {}
            for t in ("s", "t"):
                for k in range(NCH):
                    nt = nat_pool.tile([P, super_pix], F32, tag=f"nat_{t}{k}")
                    nc.sync.dma_start(
                        nt[:],
                        srcs[t][k * P : (k + 1) * P, j * super_pix : (j + 1) * super_pix],
                    )
                    nat[(t, k)] = nt
            for gl in range(GPS):
                g = j * GPS + gl
                cg = g % NCG
                oh = oh_pool.tile([P, NUM_CLASSES], F32, tag="oh")
                nc.vector.tensor_scalar(
                    oh[:], iota, lab_sb[:, g : g + 1], None, mybir.AluOpType.is_equal
                )
                for t in ("s", "t"):
                    pT = psumT_pool.tile([P, C_], F32, tag=f"pT_{t}")
                    for k in range(NCH):
                        nc.tensor.transpose(
                            pT[:, k * P : (k + 1) * P],
                            nat[(t, k)][:, gl * P : (gl + 1) * P],
                            ident,
                        )
                    sT = sbT_pool.tile([P, C_], F32, tag=f"sT_{t}")
                    if t == "s":
                        nc.vector.tensor_copy(sT[:], pT[:])
                    else:
                        nc.scalar.copy(sT[:], pT[:])
                    nc.tensor.matmul(
                        acc[t][32 * cg : 32 * cg + NUM_CLASSES, :],
                        oh[:],
                        sT[:],
                        start=(g < NCG),
                        stop=(g >= NG - NCG),
                        tile_position=(0, 32 * cg),
                        # Col-group slices write disjoint partition ranges of
                        # the same PSUM bank; the sim's zero-region group
                        # check doesn't track partition bases and trips on
                        # this legal pattern.
                        skip_group_check=True,
                    )
        for t in ("s", "t"):
            for cg in range(NCG):
                ob = outp_pool.tile([NUM_CLASSES, C_], F32, tag=f"ob_{t}{cg}")
                nc.vector.tensor_copy(
                    ob[:], acc[t][32 * cg : 32 * cg + NUM_CLASSES, :]
                )
                nc.sync.dma_start(
                    outs[t][cg * NUM_CLASSES : (cg + 1) * NUM_CLASSES, :], ob[:]
                )
    nc.finalize()
    return nc


_NC_CACHE = None


def _get_nc():
    global _NC_CACHE
    if _NC_CACHE is None:
        _NC_CACHE = build_nc()
    return _NC_CACHE


def make_misc(lab_flat, ng):
    """[identity 128 | iota 19 | labT ng] packed along the free dim."""
    labT = lab_flat.reshape(ng, P).T.astype(np.float32)
    iota = np.tile(np.arange(NUM_CLASSES, dtype=np.float32), (P, 1))
    return np.ascontiguousarray(
        np.concatenate([np.eye(P, dtype=np.float32), iota, labT], axis=1)
    )


def _make_in_maps(features_s, features_t, labels):
    in_maps = []
    for i in range(N_CORES):
        in_maps.append(
            {
                "fs": np.ascontiguousarray(features_s[i].reshape(C, HW)),
                "ft": np.ascontiguousarray(features_t[i].reshape(C, HW)),
                "misc": make_misc(labels[i].reshape(-1), HW // P),
            }
        )
    return in_maps


def _finish_on_host(results, labels):
    S_s = np.zeros((NUM_CLASSES, C), np.float64)
    S_t = np.zeros((NUM_CLASSES, C), np.float64)
    for r in results:
        S_s += r["sums_s"].reshape(NCG, NUM_CLASSES, C).sum(0)
        S_t += r["sums_t"].reshape(NCG, NUM_CLASSES, C).sum(0)
    counts = np.bincount(
        labels.reshape(-1), minlength=NUM_CLASSES
    ).astype(np.float64)
    denom = np.maximum(counts, 1.0)[:, None]

    def l2n(x):
        n = np.linalg.norm(x, axis=1, keepdims=True)
        return x / np.maximum(n, EPS)

    logits = (l2n(S_s / denom) @ l2n(S_t / denom).T) / TEMP
    m = logits.max(axis=1, keepdims=True)
    lse = m[:, 0] + np.log(np.exp(logits - m).sum(axis=1))
    per_class = np.diag(logits) - lse
    present = counts > 0
    loss = -np.sum(np.where(present, per_class, 0.0)) / np.sum(present)
    return np.asarray(loss, dtype=np.float32)


def kernel(features_s, features_t, labels, _trace=False):
    features_s = np.asarray(features_s, dtype=np.float32)
    features_t = np.asarray(features_t, dtype=np.float32)
    labels = np.asarray(labels)
    nc = _get_nc()
    in_maps = _make_in_maps(features_s, features_t, labels)
    res = run_bass_kernel_spmd(nc, in_maps, list(range(N_CORES)), trace=_trace)
    loss = _finish_on_host(res.results, labels)
    if _trace:
        return loss, res
    return loss


# revision 12
# speedup vs baseline: 1.2245x; 1.2245x over previous
"""Segment-reduce contrastive loss kernel for Trainium2 (8 NeuronCores).

Strategy (data-parallel over batch, per sharding hint):
  - Each of the 8 cores gets one batch element (fs/ft: [512, 16384] f32).
  - On-device per core: per-class channel sums for features_s/features_t
    computed as one-hot matmuls on the tensor engine. Features arrive
    channel-major, so each [128pix x 128ch] block is PE-transposed first
    (pixels must sit on the partition/contraction dim).
  - Per-class partial sums [19, 512] x2 are DMA'd out; the host sums the
    8 cores' partials (the "all-reduce"), computes counts, normalizes and
    does the tiny 19x19 contrastive logsumexp in numpy.

The segment matmuls use only 19 of 128 PE columns, so groups are
round-robined across 4 PSUM col-group slices (base partition 32*j) to get
sub-array-level concurrency; the 4 slices are summed on the host.
"""

import sys

for _p in ("/opt/trn_rl_repo",):
    if _p not in sys.path:
        sys.path.insert(0, _p)

from contextlib import ExitStack

import numpy as np

import concourse.bass as bass
import concourse.mybir as mybir
from concourse import bacc, tile
from concourse.bass_utils import run_bass_kernel_spmd

NUM_CLASSES = 19
TEMP = 0.1
EPS = 1e-12

B, C, H, W = 8, 512, 128, 128
HW = H * W
N_CORES = 8
P = 128
NCG = 4  # PSUM col-groups used round-robin by the segment matmuls
F32 = mybir.dt.float32
F32R = mybir.dt.float32r


def build_nc(C_=C, HW_=HW, super_pix=1024):
    NCH = C_ // P        # channel blocks
    NG = HW_ // P        # pixel groups of 128
    GPS = super_pix // P # groups per superchunk
    NS = HW_ // super_pix
    assert NG % NCG == 0 and NG >= 2 * NCG

    nc = bacc.Bacc()
    fs = nc.declare_dram_parameter("fs", [C_, HW_], F32, isOutput=False)
    ft = nc.declare_dram_parameter("ft", [C_, HW_], F32, isOutput=False)
    # misc: [identity 128 | iota 19 | labT NG] packed along the free dim so
    # the consts arrive in ONE DMA (multiple DMA-completion sems on one
    # consumer instruction overflow walrus's per-instruction sync slots).
    misc = nc.declare_dram_parameter("misc", [P, P + NUM_CLASSES + NG], F32, isOutput=False)
    out_s = nc.declare_dram_parameter("sums_s", [NUM_CLASSES, C_], F32, isOutput=True)
    out_t = nc.declare_dram_parameter("sums_t", [NUM_CLASSES, C_], F32, isOutput=True)

    srcs = {"s": fs, "t": ft}
    outs = {"s": out_s, "t": out_t}

    with ExitStack() as ctx:
        tc = ctx.enter_context(tile.TileContext(nc))
        const_pool = ctx.enter_context(tc.tile_pool(name="const", bufs=1))
        nat_pool = ctx.enter_context(tc.tile_pool(name="nat", bufs=3))
        psumT_pool = ctx.enter_context(tc.tile_pool(name="psumT", bufs=2, space="PSUM"))
        acc_pool = ctx.enter_context(tc.tile_pool(name="acc", bufs=1, space="PSUM"))
        sbT_pool = ctx.enter_context(tc.tile_pool(name="sbT", bufs=3))
        oh_pool = ctx.enter_context(tc.tile_pool(name="oh", bufs=4))
        outp_pool = ctx.enter_context(tc.tile_pool(name="outp", bufs=1))

        misc_sb = const_pool.tile([P, P + NUM_CLASSES + NG], F32, tag="misc")
        nc.sync.dma_start(misc_sb[:], misc[:])
        ident = misc_sb[:, 0:P]
        iota = misc_sb[:, P : P + NUM_CLASSES]
        lab_sb = misc_sb[:, P + NUM_CLASSES : P + NUM_CLASSES + NG]

        acc = {
            t: acc_pool.tile([P, C_], F32, tag=f"acc_{t}", name=f"acc_{t}")
            for t in ("s", "t")
        }

        # Warm-up transpose reading only the const tile: pre-pays the misc
        # DMA wait on PE, so the first real transpose needs just one wait
        # (walrus allows a single embedded sync-wait per instruction).
        warm = psumT_pool.tile([P, P], F32, tag="pT_s", name="warm")
        nc.tensor.transpose(warm[:, 0:P], ident, ident)

        pend = []

        def _mm(item):
            # fp32r matmuls reject non-zero col-group tile_position, so all
            # groups accumulate into partition rows 0..18 of each bank; at
            # 1 cycle/row the lost sub-array concurrency is cheap.
            g, t, oh, sT = item
            nc.tensor.matmul(
                acc[t][0:NUM_CLASSES, :],
                oh[:],
                sT[:],
                start=(g == 0),
                stop=(g == NG - 1),
            )

        for j in range(NS):
            nat = {}
            for t in ("s", "t"):
                for k in range(NCH):
                    nt = nat_pool.tile([P, super_pix], F32, tag=f"nat_{t}{k}")
                    nc.sync.dma_start(
                        nt[:],
                        srcs[t][k * P : (k + 1) * P, j * super_pix : (j + 1) * super_pix],
                    )
                    nat[(t, k)] = nt
            for gl in range(GPS):
                g = j * GPS + gl
                oh = oh_pool.tile([P, NUM_CLASSES], F32R, tag="oh")
                nc.vector.tensor_scalar(
                    oh[:], iota, lab_sb[:, g : g + 1], None, mybir.AluOpType.is_equal
                )
                for t in ("s", "t"):
                    pT = psumT_pool.tile([P, C_], F32, tag=f"pT_{t}")
                    for k in range(NCH):
                        nc.tensor.transpose(
                            pT[:, k * P : (k + 1) * P],
                            nat[(t, k)][:, gl * P : (gl + 1) * P],
                            ident,
                        )
                    # fp32r output: rounds for the fp32r segment matmul
                    # (1 cycle/row vs fp32's 4).
                    sT = sbT_pool.tile([P, C_], F32R, tag=f"sT_{t}")
                    if t == "s":
                        nc.vector.tensor_copy(sT[:], pT[:])
                    else:
                        nc.scalar.copy(sT[:], pT[:])
                    pend.append((g, t, oh, sT))
                # Emit segment matmuls one group late so the in-order PE can
                # run group g+1's transposes while group g's PSUM->SBUF copies
                # complete (otherwise every matmul stalls on its copy).
                while len(pend) > 2:
                    _mm(pend.pop(0))
        while pend:
            _mm(pend.pop(0))
        for t in ("s", "t"):
            ob = outp_pool.tile([NUM_CLASSES, C_], F32, tag=f"ob_{t}", name=f"ob_{t}")
            nc.vector.tensor_copy(ob[:], acc[t][0:NUM_CLASSES, :])
            nc.sync.dma_start(outs[t][:], ob[:])
    nc.finalize()
    return nc


_NC_CACHE = None


def _get_nc():
    global _NC_CACHE
    if _NC_CACHE is None:
        _NC_CACHE = build_nc()
    return _NC_CACHE


def make_misc(lab_flat, ng):
    """[identity 128 | iota 19 | labT ng] packed along the free dim."""
    labT = lab_flat.reshape(ng, P).T.astype(np.float32)
    iota = np.tile(np.arange(NUM_CLASSES, dtype=np.float32), (P, 1))
    return np.ascontiguousarray(
        np.concatenate([np.eye(P, dtype=np.float32), iota, labT], axis=1)
    )


def _make_in_maps(features_s, features_t, labels):
    in_maps = []
    for i in range(N_CORES):
        in_maps.append(
            {
                "fs": np.ascontiguousarray(features_s[i].reshape(C, HW)),
                "ft": np.ascontiguousarray(features_t[i].reshape(C, HW)),
                "misc": make_misc(labels[i].reshape(-1), HW // P),
            }
        )
    return in_maps


def _finish_on_host(results, labels):
    S_s = np.zeros((NUM_CLASSES, C), np.float64)
    S_t = np.zeros((NUM_CLASSES, C), np.float64)
    for r in results:
        S_s += r["sums_s"]
        S_t += r["sums_t"]
    counts = np.bincount(
        labels.reshape(-1), minlength=NUM_CLASSES
    ).astype(np.float64)
    denom = np.maximum(counts, 1.0)[:, None]

    def l2n(x):
        n = np.linalg.norm(x, axis=1, keepdims=True)
        return x / np.maximum(n, EPS)

    logits = (l2n(S_s / denom) @ l2n(S_t / denom).T) / TEMP
    m = logits.max(axis=1, keepdims=True)
    lse = m[:, 0] + np.log(np.exp(logits - m).sum(axis=1))
    per_class = np.diag(logits) - lse
    present = counts > 0
    loss = -np.sum(np.where(present, per_class, 0.0)) / np.sum(present)
    return np.asarray(loss, dtype=np.float32)


def kernel(features_s, features_t, labels, _trace=False):
    features_s = np.asarray(features_s, dtype=np.float32)
    features_t = np.asarray(features_t, dtype=np.float32)
    labels = np.asarray(labels)
    nc = _get_nc()
    in_maps = _make_in_maps(features_s, features_t, labels)
    res = run_bass_kernel_spmd(nc, in_maps, list(range(N_CORES)), trace=_trace)
    loss = _finish_on_host(res.results, labels)
    if _trace:
        return loss, res
    return loss


# revision 13
# speedup vs baseline: 1.5270x; 1.2470x over previous
"""Segment-reduce contrastive loss kernel for Trainium2 (8 NeuronCores).

Strategy (data-parallel over batch, per sharding hint):
  - Each of the 8 cores gets one batch element (fs/ft: [512, 16384] f32).
  - On-device per core: per-class channel sums for features_s/features_t
    computed as one-hot matmuls on the tensor engine. Features arrive
    channel-major, so each [128pix x 128ch] block is PE-transposed first
    (pixels must sit on the partition/contraction dim).
  - Per-class partial sums [19, 512] x2 are DMA'd out; the host sums the
    8 cores' partials (the "all-reduce"), computes counts, normalizes and
    does the tiny 19x19 contrastive logsumexp in numpy.

The segment matmuls use only 19 of 128 PE columns, so groups are
round-robined across 4 PSUM col-group slices (base partition 32*j) to get
sub-array-level concurrency; the 4 slices are summed on the host.
"""

import sys

for _p in ("/opt/trn_rl_repo",):
    if _p not in sys.path:
        sys.path.insert(0, _p)

from contextlib import ExitStack

import numpy as np

import concourse.bass as bass
import concourse.mybir as mybir
from concourse import bacc, tile
from concourse.bass_utils import run_bass_kernel_spmd

NUM_CLASSES = 19
TEMP = 0.1
EPS = 1e-12

B, C, H, W = 8, 512, 128, 128
HW = H * W
N_CORES = 8
P = 128
NCG = 4  # PSUM col-groups used round-robin by the segment matmuls
F32 = mybir.dt.float32
F32R = mybir.dt.float32r


def build_nc(C_=C, HW_=HW, super_pix=1024):
    NCH = C_ // P        # channel blocks
    NG = HW_ // P        # pixel groups of 128
    GPS = super_pix // P # groups per superchunk
    NS = HW_ // super_pix
    assert NG % NCG == 0 and NG >= 2 * NCG

    nc = bacc.Bacc()
    fs = nc.declare_dram_parameter("fs", [C_, HW_], F32, isOutput=False)
    ft = nc.declare_dram_parameter("ft", [C_, HW_], F32, isOutput=False)
    # misc: [identity 128 | iota 19 | labT NG] packed along the free dim so
    # the consts arrive in ONE DMA (multiple DMA-completion sems on one
    # consumer instruction overflow walrus's per-instruction sync slots).
    misc = nc.declare_dram_parameter("misc", [P, P + NUM_CLASSES + NG], F32, isOutput=False)
    out_s = nc.declare_dram_parameter("sums_s", [NUM_CLASSES, C_], F32, isOutput=True)
    out_t = nc.declare_dram_parameter("sums_t", [NUM_CLASSES, C_], F32, isOutput=True)

    srcs = {"s": fs, "t": ft}
    outs = {"s": out_s, "t": out_t}

    with ExitStack() as ctx:
        tc = ctx.enter_context(tile.TileContext(nc))
        const_pool = ctx.enter_context(tc.tile_pool(name="const", bufs=1))
        nat_pool = ctx.enter_context(tc.tile_pool(name="nat", bufs=3))
        psumT_pool = ctx.enter_context(tc.tile_pool(name="psumT", bufs=3, space="PSUM"))
        acc_pool = ctx.enter_context(tc.tile_pool(name="acc", bufs=1, space="PSUM"))
        sbT_pool = ctx.enter_context(tc.tile_pool(name="sbT", bufs=5))
        oh_pool = ctx.enter_context(tc.tile_pool(name="oh", bufs=6))
        outp_pool = ctx.enter_context(tc.tile_pool(name="outp", bufs=1))

        misc_sb = const_pool.tile([P, P + NUM_CLASSES + NG], F32, tag="misc")
        nc.sync.dma_start(misc_sb[:], misc[:])
        ident = misc_sb[:, 0:P]
        iota = misc_sb[:, P : P + NUM_CLASSES]
        lab_sb = misc_sb[:, P + NUM_CLASSES : P + NUM_CLASSES + NG]

        acc = {
            t: acc_pool.tile([P, C_], F32, tag=f"acc_{t}", name=f"acc_{t}")
            for t in ("s", "t")
        }

        # Warm-up transpose reading only the const tile: pre-pays the misc
        # DMA wait on PE, so the first real transpose needs just one wait
        # (walrus allows a single embedded sync-wait per instruction).
        warm = psumT_pool.tile([P, P], F32, tag="pT_s", name="warm")
        nc.tensor.transpose(warm[:, 0:P], ident, ident)

        pend = []

        def _mm(item):
            # fp32r matmuls reject non-zero col-group tile_position, so all
            # groups accumulate into partition rows 0..18 of each bank; at
            # 1 cycle/row the lost sub-array concurrency is cheap.
            g, t, oh, sT = item
            nc.tensor.matmul(
                acc[t][0:NUM_CLASSES, :],
                oh[:],
                sT[:],
                start=(g == 0),
                stop=(g == NG - 1),
            )

        for j in range(NS):
            nat = {}
            for t in ("s", "t"):
                for k in range(NCH):
                    nt = nat_pool.tile([P, super_pix], F32, tag=f"nat_{t}{k}")
                    nc.sync.dma_start(
                        nt[:],
                        srcs[t][k * P : (k + 1) * P, j * super_pix : (j + 1) * super_pix],
                    )
                    nat[(t, k)] = nt
            for gl in range(GPS):
                g = j * GPS + gl
                oh = oh_pool.tile([P, NUM_CLASSES], F32R, tag="oh")
                nc.vector.tensor_scalar(
                    oh[:], iota, lab_sb[:, g : g + 1], None, mybir.AluOpType.is_equal
                )
                for t in ("s", "t"):
                    pT = psumT_pool.tile([P, C_], F32, tag=f"pT_{t}")
                    for k in range(NCH):
                        nc.tensor.transpose(
                            pT[:, k * P : (k + 1) * P],
                            nat[(t, k)][:, gl * P : (gl + 1) * P],
                            ident,
                        )
                    # fp32r output: rounds for the fp32r segment matmul
                    # (1 cycle/row vs fp32's 4).
                    sT = sbT_pool.tile([P, C_], F32R, tag=f"sT_{t}")
                    if t == "s":
                        nc.vector.tensor_copy(sT[:], pT[:])
                    else:
                        nc.scalar.copy(sT[:], pT[:])
                    pend.append((g, t, oh, sT))
                # Emit segment matmuls one group late so the in-order PE can
                # run group g+1's transposes while group g's PSUM->SBUF copies
                # complete (otherwise every matmul stalls on its copy).
                while len(pend) > 4:
                    _mm(pend.pop(0))
        while pend:
            _mm(pend.pop(0))
        for t in ("s", "t"):
            ob = outp_pool.tile([NUM_CLASSES, C_], F32, tag=f"ob_{t}", name=f"ob_{t}")
            nc.vector.tensor_copy(ob[:], acc[t][0:NUM_CLASSES, :])
            nc.sync.dma_start(outs[t][:], ob[:])
    nc.finalize()
    return nc


_NC_CACHE = None


def _get_nc():
    global _NC_CACHE
    if _NC_CACHE is None:
        _NC_CACHE = build_nc()
    return _NC_CACHE


def make_misc(lab_flat, ng):
    """[identity 128 | iota 19 | labT ng] packed along the free dim."""
    labT = lab_flat.reshape(ng, P).T.astype(np.float32)
    iota = np.tile(np.arange(NUM_CLASSES, dtype=np.float32), (P, 1))
    return np.ascontiguousarray(
        np.concatenate([np.eye(P, dtype=np.float32), iota, labT], axis=1)
    )


def _make_in_maps(features_s, features_t, labels):
    in_maps = []
    for i in range(N_CORES):
        in_maps.append(
            {
                "fs": np.ascontiguousarray(features_s[i].reshape(C, HW)),
                "ft": np.ascontiguousarray(features_t[i].reshape(C, HW)),
                "misc": make_misc(labels[i].reshape(-1), HW // P),
            }
        )
    return in_maps


def _finish_on_host(results, labels):
    S_s = np.zeros((NUM_CLASSES, C), np.float64)
    S_t = np.zeros((NUM_CLASSES, C), np.float64)
    for r in results:
        S_s += r["sums_s"]
        S_t += r["sums_t"]
    counts = np.bincount(
        labels.reshape(-1), minlength=NUM_CLASSES
    ).astype(np.float64)
    denom = np.maximum(counts, 1.0)[:, None]

    def l2n(x):
        n = np.linalg.norm(x, axis=1, keepdims=True)
        return x / np.maximum(n, EPS)

    logits = (l2n(S_s / denom) @ l2n(S_t / denom).T) / TEMP
    m = logits.max(axis=1, keepdims=True)
    lse = m[:, 0] + np.log(np.exp(logits - m).sum(axis=1))
    per_class = np.diag(logits) - lse
    present = counts > 0
    loss = -np.sum(np.where(present, per_class, 0.0)) / np.sum(present)
    return np.asarray(loss, dtype=np.float32)


def kernel(features_s, features_t, labels, _trace=False):
    features_s = np.asarray(features_s, dtype=np.float32)
    features_t = np.asarray(features_t, dtype=np.float32)
    labels = np.asarray(labels)
    nc = _get_nc()
    in_maps = _make_in_maps(features_s, features_t, labels)
    res = run_bass_kernel_spmd(nc, in_maps, list(range(N_CORES)), trace=_trace)
    loss = _finish_on_host(res.results, labels)
    if _trace:
        return loss, res
    return loss
